# revision 34
# baseline (speedup 1.0000x reference)
"""Multi-head attention block (QKV proj -> softmax attention -> out proj) for
Trainium2, SPMD across 8 NeuronCores.

Sharding: batch (B=2) x head-groups (4 groups of 4 heads). Core c handles
batch c//4 and heads [4*(c%4), 4*(c%4)+4). Each core computes its partial
output contribution (context @ wo_slice.T); the host sums the 4 head-group
partials per batch (tensor-parallel row-sharded wo => the all-reduce is the
host-side gather).

Single head-granular software pipeline: the PE interleaves QKV projection
chains of head h+1 (and PV blocks of head h / h-1, out-proj at the end)
between the scores units of head h, so the scalar-exp / DVE-normalize /
x-bar-transpose softmax pipeline never stalls the PE. Softmax is standard
[query, key] orientation: scalar exp writes probs AND row sums via accum_out,
DVE normalizes in place, DMA x-bar transposes to [key, query] for PV.

PSUM (8 banks): scores 2x[128,1024], QKV/out-proj chains 3x[128,512],
PV 1x[128,512]. wo tiles alias 4 freed x tiles (same pool tags) to fit SBUF.
Partial outputs are stored bf16 and summed in f32 on the host.
"""

import sys

if "/opt/trn_rl_repo" not in sys.path:
    sys.path.insert(0, "/opt/trn_rl_repo")

from contextlib import ExitStack

import ml_dtypes
import numpy as np

import concourse.bacc as bacc
import concourse.tile as tile
from concourse import mybir
from concourse.bass_utils import run_bass_kernel_spmd

BF16 = mybir.dt.bfloat16
F32 = mybir.dt.float32

B, S, DIM = 2, 2048, 2048
HEADS, HD = 16, 128
P = 128
N_CORES = 8
HGROUPS = 4  # head groups (second shard axis is batch)
HPC = HEADS // HGROUPS  # heads per core = 4
DL = HPC * HD  # local head dims per core = 512
SCALE = 1.0 / float(np.sqrt(HD))

NK = DIM // P  # 16 contraction tiles for the projections
NMT = S // P  # 16 query tiles of 128 tokens
NNT = S // P  # 16 kv tiles of 128 tokens

_PROGRAM_CACHE = {}


def _emit_fast(nc, tc, xT, wq_arr, wk_arr, wv_arr, woT, out):
    """Pipelined no-mask variant."""
    with ExitStack() as octx:
        # ---- persistent SBUF pools ----
        xpool = octx.enter_context(tc.tile_pool(name="xt", bufs=1))
        xts = [xpool.tile([P, S], BF16, tag=f"x{kt}", name=f"x{kt}")
               for kt in range(NK)]
        wpools = {
            name: octx.enter_context(tc.tile_pool(name=f"w{name}", bufs=2))
            for name in ("q", "k", "v")
        }
        qpool = octx.enter_context(tc.tile_pool(name="qp", bufs=2))
        kpool = octx.enter_context(tc.tile_pool(name="kp", bufs=2))
        vTpool = octx.enter_context(tc.tile_pool(name="vTp", bufs=1))
        vvpool = octx.enter_context(tc.tile_pool(name="vvp", bufs=2))
        ctxpool = octx.enter_context(tc.tile_pool(name="ctxp", bufs=1))
        ctx_sb = [ctxpool.tile([P, S], BF16, tag=f"ctx{h}", name=f"ctx{h}")
                  for h in range(HPC)]
        pbm_pool = octx.enter_context(tc.tile_pool(name="pbm", bufs=5))
        pbt_pool = octx.enter_context(tc.tile_pool(name="pbt", bufs=3))
        stats = octx.enter_context(tc.tile_pool(name="stats", bufs=8))
        recp = octx.enter_context(tc.tile_pool(name="recp", bufs=16))
        ob_pool = octx.enter_context(tc.tile_pool(name="ob", bufs=6))

        # ---- PSUM pools: 4 + 3 + 1 banks ----
        ps_sc = octx.enter_context(
            tc.tile_pool(name="ps_sc", bufs=2, space="PSUM"))
        ps_qkv = octx.enter_context(
            tc.tile_pool(name="ps_qkv", bufs=3, space="PSUM"))
        ps_pv = octx.enter_context(
            tc.tile_pool(name="ps_pv", bufs=1, space="PSUM"))

        w_srcs = {"q": wq_arr, "k": wk_arr, "v": wv_arr}
        w_tiles = {}  # (name, h) -> tile
        q_sb = {}     # h -> [P, S] tile
        k_sb = {}
        vT_sb = {}
        vvs = {}
        pbts = {}     # (h, jb) -> tile
        wo_sb = {}    # h -> tile (aliases xts[h])

        def load_w(name, h):
            t = wpools[name].tile([P, NK, P], BF16, tag=f"w{name}",
                                  name=f"w{name}{h}")
            w_tiles[(name, h)] = t
            nc.gpsimd.dma_start(t[:], w_srcs[name][h])

        def load_wo(h):
            """wo tiles alias freed buffers: h<3 alias the w pools' spare
            buffer (freed after head-2 chains), h=3 aliases xts[0] (freed
            after the last slot-3 QKV chain)."""
            if h < 3:
                name = ("k", "q", "v")[h]
                t = wpools[name].tile([P, NK, P], BF16, tag=f"w{name}",
                                      name=f"wo{h}")
                src = woT[h * P: (h + 1) * P, :].rearrange(
                    "p (k c) -> p k c", k=NK)
            else:
                t = xpool.tile([P, S], BF16, tag="x0", name="wo3")
                src = woT[h * P: (h + 1) * P, :]
            wo_sb[h] = t
            nc.gpsimd.dma_start(t[:], src)

        def wo_slice(h, ec):
            t = wo_sb[h]
            if h < 3:
                return t[:, 4 * ec: 4 * (ec + 1), :]
            return t[:, ec * 512: (ec + 1) * 512]

        # ---- upfront DMA issue ----
        # ALL early loads go on gpsimd IN NEED ORDER: concurrent queues
        # share HBM bandwidth, so fanning out just slows the critical first
        # tiles -- a single in-order queue is the priority mechanism. The
        # prime's kt-major chains track the x stream tile by tile. Only
        # heads 0/1 load up front; head h+2's loads are emitted inside
        # slot h right after head h's chains so the pool-rotation WAR
        # waits never block the gpsimd queue.
        load_w("k", 0)
        load_w("q", 0)
        for kt in range(NK):
            # split each tile across two queues (half the partitions each)
            # so the earliest tiles land ~2x sooner for the prime
            nc.gpsimd.dma_start(xts[kt][0:64, :], xT[kt * P: kt * P + 64, :])
            nc.sync.dma_start(xts[kt][64:128, :],
                              xT[kt * P + 64: (kt + 1) * P, :])
        load_w("v", 0)
        for name in ("k", "q", "v"):
            load_w(name, 1)

        # ---- work-item emitters ----
        def chain(name, h, quarter):
            """One QKV projection chain: 16 matmuls -> one 512-token quarter
            of q/k/vT for head h."""
            if name == "q":
                if h not in q_sb:
                    q_sb[h] = qpool.tile([P, S], BF16, tag="q", name=f"q{h}")
                dst = q_sb[h]
            elif name == "k":
                if h not in k_sb:
                    k_sb[h] = kpool.tile([P, S], BF16, tag="k", name=f"k{h}")
                dst = k_sb[h]
            else:
                if h not in vT_sb:
                    vT_sb[h] = vTpool.tile([P, S], BF16, tag="vT",
                                           name=f"vT{h}")
                dst = vT_sb[h]
            w = w_tiles[(name, h)]
            ps = ps_qkv.tile([P, 512], F32, tag="ps_qkv")
            for kt in range(NK):
                nc.tensor.matmul(
                    ps[:],
                    w[:, kt, :],
                    xts[kt][:, quarter * 512: (quarter + 1) * 512],
                    start=(kt == 0),
                    stop=(kt == NK - 1),
                )
            nc.vector.tensor_copy(
                dst[:, quarter * 512: (quarter + 1) * 512], ps[:])
            if name == "v" and quarter == 3:
                # whole vT tile ready -> transpose to [kv, d] for PV
                vvs[h] = vvpool.tile([P, NNT, P], BF16, tag="vv",
                                     name=f"vv{h}")
                nc.sync.dma_start(vvs[h][:], vT_sb[h][:], transpose=True)

        def scores_unit(h, u):
            """One 128-query scores unit: 4 matmuls + exp/accum + den/recip
            + in-place normalize + x-bar transpose into pbt[(h, u//4)]."""
            jb, mtl = u // 4, u % 4
            if (h, jb) not in pbts:
                pbts[(h, jb)] = pbt_pool.tile([P, NNT, 4, P], BF16,
                                              tag="pbt", name="pbt")
            pbt = pbts[(h, jb)]
            qt = q_sb[h][:, u * P: (u + 1) * P]
            pbm = pbm_pool.tile([P, S], BF16, tag="pbm", name="pbm")
            for c in range(2):
                ps = ps_sc.tile([P, 1024], F32, tag="ps_sc")
                for sub in range(2):
                    k0 = c * 1024 + sub * 512
                    nc.tensor.matmul(
                        ps[:, sub * 512: (sub + 1) * 512],
                        qt,
                        k_sb[h][:, k0: k0 + 512],
                        start=True,
                        stop=True,
                    )
                nc.scalar.activation(
                    pbm[:, c * 1024: (c + 1) * 1024],
                    ps[:],
                    mybir.ActivationFunctionType.Exp,
                    scale=SCALE,
                )
            # row sums on the DVE (keeps the scalar engine exp-only; the
            # accum_out + ACTIVATION_READ_ACCUMULATOR path cost 0.6us/unit
            # of scalar time, which paces slots 0 and 3)
            den = stats.tile([P, 1], F32, tag="den", name="den")
            nc.vector.tensor_reduce(
                den[:], pbm[:], mybir.AxisListType.X, mybir.AluOpType.add)
            rec = recp.tile([P, 1], F32, tag="rec", name="rec")
            nc.vector.reciprocal(rec[:], den[:])
            nc.vector.tensor_scalar_mul(pbm[:], pbm[:], rec[:])
            nc.sync.dma_start(pbt[:, :, mtl, :], pbm[:], transpose=True)

        def pv_block(h, jb):
            """ctx[h][:, jb*512:(jb+1)*512] = V_h.T @ probs block."""
            pbt = pbts.pop((h, jb))
            ps = ps_pv.tile([P, 512], F32, tag="ps_pv")
            for nt in range(NNT):
                nc.tensor.matmul(
                    ps[:],
                    vvs[h][:, nt, :],
                    pbt[:, nt, :, :],
                    start=(nt == 0),
                    stop=(nt == NNT - 1),
                )
            nc.vector.tensor_copy(
                ctx_sb[h][:, jb * 512: (jb + 1) * 512], ps[:])

        op_count = [0]

        def outproj_chain(tt, ec):
            """out[tt, ec*512:...] partial: chain over the 4 heads."""
            i0 = op_count[0]
            if i0 >= 28 and i0 % 2 == 1:
                # tail: alternate into the scores psum banks (free once the
                # last exp has read them) to deepen the copy-WAR rotation
                pst = ps_sc.tile([P, 1024], F32, tag="ps_sc", name="ps_op")
                ps = pst[:, 0:512]
            else:
                ps = ps_qkv.tile([P, 512], F32, tag="ps_qkv")
            for h in range(HPC):
                nc.tensor.matmul(
                    ps[:],
                    ctx_sb[h][:, tt * P: (tt + 1) * P],
                    wo_slice(h, ec),
                    start=(h == 0),
                    stop=(h == HPC - 1),
                )
            i = op_count[0]
            op_count[0] += 1
            ob = ob_pool.tile([P, 512], BF16, tag="ob", name="ob")
            # The first 12 chains run inside slot 3 while the scalar engine
            # still streams exps -- keep its queue clear (DVE copy, gpsimd
            # store). In the tail, alternate copies DVE/scalar and stores
            # gpsimd/sync so no single queue paces the drain.
            # scalar joins the copy rotation only once the slot-3 exp
            # stream (which shares its queue) has surely drained
            if i < 28 or i % 2 == 0:
                nc.vector.tensor_copy(ob[:], ps[:])
            else:
                nc.scalar.copy(ob[:], ps[:])
            if i < 12:
                store_eng = nc.gpsimd
            else:
                store_eng = nc.gpsimd if i % 2 == 0 else nc.sync
            store_eng.dma_start(
                out[tt * P: (tt + 1) * P, ec * 512: (ec + 1) * 512], ob[:])

        # ---- schedule ----
        # Prime: k0..k3, q0, q1 of head 0 emitted kt-major as six concurrent
        # accumulation chains (3 on ps_qkv banks, 3 on the two ps_sc tiles'
        # 512-halves) so the PE tracks the x DMA stream tile-by-tile instead
        # of idling until the whole 8MB lands.
        q_sb[0] = qpool.tile([P, S], BF16, tag="q", name="q0")
        k_sb[0] = kpool.tile([P, S], BF16, tag="k", name="k0")
        pr_qkv = [ps_qkv.tile([P, 512], F32, tag="ps_qkv", name=f"prq{i}")
                  for i in range(3)]
        pr_sc = [ps_sc.tile([P, 1024], F32, tag="ps_sc", name=f"prs{i}")
                 for i in range(2)]
        # (dst, quarter, psum AP)
        prime = [
            (k_sb[0], 0, pr_qkv[0][:]),
            (k_sb[0], 1, pr_qkv[1][:]),
            (q_sb[0], 0, pr_qkv[2][:]),
            (k_sb[0], 2, pr_sc[0][:, 0:512]),
            (k_sb[0], 3, pr_sc[0][:, 512:1024]),
            (q_sb[0], 1, pr_sc[1][:, 0:512]),
        ]
        wk0 = w_tiles[("k", 0)]
        wq0 = w_tiles[("q", 0)]
        for kt in range(NK):
            for dst, quarter, ps in prime:
                nc.tensor.matmul(
                    ps,
                    (wq0 if dst is q_sb[0] else wk0)[:, kt, :],
                    xts[kt][:, quarter * 512: (quarter + 1) * 512],
                    start=(kt == 0),
                    stop=(kt == NK - 1),
                )
        for dst, quarter, ps in prime:
            nc.vector.tensor_copy(
                dst[:, quarter * 512: (quarter + 1) * 512], ps)

        # Fillers per slot h, keyed by unit index. Steady-state slots carry:
        # rest-of-h chains (q1..q3, v0..v3), next head's k0..k3 + q0, PV of
        # (h-1, jb2/jb3) and (h, jb0/jb1). Head h+2's weight DMAs are
        # re-emitted just after head h's chains so their pool-rotation waits
        # resolve instantly. Slot 3 replaces next-head chains with out-proj
        # of jb0 and pulls its own v chains earlier so vv(h3) beats
        # PV(h3, jb0).
        op_queue = [(tt, ec) for tt in range(NMT) for ec in range(4)]

        def take_op(n):
            for _ in range(n):
                if op_queue:
                    outproj_chain(*op_queue.pop(0))

        for h in range(HPC):
            last = h == HPC - 1
            if not last:
                fillers = {
                    0: [("loadw", "k", h + 2), ("chain", "q", h, 1)],
                    1: [("chain", "v", h, 0)],
                    2: [("pv", h - 1, 2)],
                    3: [("chain", "v", h, 1)],
                    4: [("chain", "v", h, 2)],
                    5: [("pv", h - 1, 3)],
                    6: [("chain", "q", h, 2)],
                    7: [("chain", "v", h, 3)],
                    8: [("chain", "q", h, 3), ("loadw", "v", h + 2)],
                    9: [("chain", "k", h + 1, 0), ("loadw", "q", h + 2)],
                    10: [("chain", "k", h + 1, 1)],
                    11: [("chain", "k", h + 1, 2)],
                    12: [("pv", h, 0)],
                    13: [("chain", "k", h + 1, 3)],
                    14: [("pv", h, 1)],
                    15: [("chain", "q", h + 1, 0)],
                }
                if h == 0:
                    del fillers[0][1]  # q1(h0) ran in the prime
                if h == 2:
                    # wo0..wo2 alias w-pool buffers freed by head-2's chains
                    fillers[0].append(("loadwo", 0))
                    fillers[8].append(("loadwo", 2))
                    fillers[9].append(("loadwo", 1))
            else:
                # v chains front-loaded so the vv transpose queues ahead of
                # most pbm transposes and beats PV(h3, 0) at u10
                fillers = {
                    0: [("chain", "q", h, 1)],
                    1: [("chain", "v", h, 0)],
                    2: [("chain", "v", h, 1)],
                    3: [("pv", h - 1, 2)],
                    4: [("chain", "v", h, 2)],
                    5: [("chain", "v", h, 3)],
                    6: [("chain", "q", h, 2)],
                    7: [("pv", h - 1, 3)],
                    8: [("chain", "q", h, 3), ("loadwo", 3)],
                    10: [("pv", h, 0)],
                    11: [("op", 3)],
                    12: [("op", 3)],
                    13: [("pv", h, 1)],
                    14: [("op", 3)],
                    15: [("op", 3)],
                }
            for u in range(16):
                scores_unit(h, u)
                for item in fillers.get(u, []):
                    if item[0] == "chain":
                        chain(item[1], item[2], item[3])
                    elif item[0] == "pv":
                        if item[1] >= 0:
                            pv_block(item[1], item[2])
                    elif item[0] == "loadw":
                        if item[2] < HPC:
                            load_w(item[1], item[2])
                    elif item[0] == "loadwo":
                        load_wo(item[1])
                    else:
                        take_op(item[1])

        # ---- tail ----
        # Ready op chains go BEFORE each PV block: the PE queue is in-order,
        # so a PV waiting on slot-3's exp-paced transposes must not block
        # chains whose inputs are already complete.
        take_op(16)
        pv_block(3, 2)
        take_op(16)
        pv_block(3, 3)
        take_op(len(op_queue))


def _emit_masked(nc, tc, xT, wqT, wkT, wvT, woT, maskf, out):
    """Two-phase variant with additive mask (baseline path; mask is zero in
    the benchmark so this is correctness-only)."""
    with ExitStack() as octx:
        planes = octx.enter_context(tc.tile_pool(name="planes", bufs=1))
        q_sb = [planes.tile([P, S], BF16, tag=f"q{h}", name=f"q{h}") for h in range(HPC)]
        k_sb = [planes.tile([P, S], BF16, tag=f"k{h}", name=f"k{h}") for h in range(HPC)]
        ctx_sb = [planes.tile([P, S], BF16, tag=f"ctx{h}", name=f"ctx{h}")
                  for h in range(HPC)]
        vv_pool = octx.enter_context(tc.tile_pool(name="vv", bufs=1))
        vvs = [vv_pool.tile([P, NNT, P], BF16, tag=f"vv{h}", name=f"vv{h}")
               for h in range(HPC)]

        ps_sc = octx.enter_context(tc.tile_pool(name="ps_sc", bufs=3, space="PSUM"))
        ps_small = octx.enter_context(
            tc.tile_pool(name="ps_small", bufs=2, space="PSUM")
        )

        with ExitStack() as ctx:
            wpool = ctx.enter_context(tc.tile_pool(name="wqkv", bufs=1))
            w_sb = {}
            for name, src in (("q", wqT), ("k", wkT), ("v", wvT)):
                w_sb[name] = wpool.tile([P, NK, DL], BF16, tag=f"w{name}",
                                        name=f"w{name}")
            vT_sb = [wpool.tile([P, S], BF16, tag=f"vt{h}", name=f"vt{h}")
                     for h in range(HPC)]
            xpool = ctx.enter_context(tc.tile_pool(name="xt", bufs=1))
            xts = [xpool.tile([P, S], BF16, tag=f"x{kt}", name=f"x{kt}")
                   for kt in range(NK)]
            for kt in range(NK):
                nc.sync.dma_start(xts[kt][:], xT[kt * P : (kt + 1) * P, :])
            for name, src in (("q", wqT), ("k", wkT), ("v", wvT)):
                for kt in range(NK):
                    nc.gpsimd.dma_start(
                        w_sb[name][:, kt, :], src[kt * P : (kt + 1) * P, :]
                    )

            for h in range(HPC):
                for name, dsts in (("q", q_sb), ("k", k_sb), ("v", vT_sb)):
                    for half in range(2):
                        ps = ps_sc.tile([P, 1024], F32, tag="ps_sc")
                        for kt in range(NK):
                            for mc in range(2):
                                m0 = half * 1024 + mc * 512
                                nc.tensor.matmul(
                                    ps[:, mc * 512 : (mc + 1) * 512],
                                    w_sb[name][:, kt, h * P : (h + 1) * P],
                                    xts[kt][:, m0 : m0 + 512],
                                    start=(kt == 0),
                                    stop=(kt == NK - 1),
                                )
                        nc.any.tensor_copy(
                            dsts[h][:, half * 1024 : (half + 1) * 1024], ps[:]
                        )
                nc.sync.dma_start(vvs[h][:], vT_sb[h][:], transpose=True)

        with ExitStack() as ctx:
            wopool = ctx.enter_context(tc.tile_pool(name="wo", bufs=1))
            wo_sb = [wopool.tile([P, DIM], BF16, tag=f"wo{h}", name=f"wo{h}")
                     for h in range(HPC)]
            for h in range(HPC):
                nc.gpsimd.dma_start(wo_sb[h][:], woT[h * P : (h + 1) * P, :])

            pbm_pool = ctx.enter_context(tc.tile_pool(name="pbm", bufs=14))
            pbt_pool = ctx.enter_context(tc.tile_pool(name="pbt", bufs=4))
            stats = ctx.enter_context(tc.tile_pool(name="stats", bufs=8))
            recp = ctx.enter_context(tc.tile_pool(name="recp", bufs=16))
            ob_pool = ctx.enter_context(tc.tile_pool(name="ob", bufs=3))
            mpool = ctx.enter_context(tc.tile_pool(name="mask", bufs=3))

            def scores_unit(h, jb, mtl, pbt_dst):
                mt = 4 * jb + mtl
                qt = q_sb[h][:, mt * P : (mt + 1) * P]
                pbm = pbm_pool.tile([P, S], BF16, tag="pbm", name="pbm")
                accs = stats.tile([P, 2], F32, tag="accs", name="accs")
                mts = mpool.tile([P, S], F32, tag="mt", name="mts")
                nc.gpsimd.dma_start(mts[:], maskf[mt * P : (mt + 1) * P, :])
                for c in range(2):
                    ps = ps_sc.tile([P, 1024], F32, tag="ps_sc")
                    for sub in range(2):
                        k0 = c * 1024 + sub * 512
                        nc.tensor.matmul(
                            ps[:, sub * 512 : (sub + 1) * 512],
                            qt,
                            k_sb[h][:, k0 : k0 + 512],
                            start=True,
                            stop=True,
                        )
                    nc.vector.tensor_add(
                        ps[:], ps[:], mts[:, c * 1024 : (c + 1) * 1024]
                    )
                    nc.scalar.activation(
                        pbm[:, c * 1024 : (c + 1) * 1024],
                        ps[:],
                        mybir.ActivationFunctionType.Exp,
                        scale=SCALE,
                        accum_out=accs[:, c : c + 1],
                    )
                den = stats.tile([P, 1], F32, tag="den", name="den")
                nc.vector.tensor_add(den[:], accs[:, 0:1], accs[:, 1:2])
                rec = recp.tile([P, 1], F32, tag="rec", name="rec")
                nc.vector.reciprocal(rec[:], den[:])
                nc.vector.tensor_scalar_mul(pbm[:], pbm[:], rec[:])
                nc.sync.dma_start(
                    pbt_dst[:, :, mtl, :], pbm[:], transpose=True
                )

            def make_pv(h, jb, pbt):
                def pv():
                    ps = ps_small.tile([P, 512], F32, tag="ps_small")
                    for nt in range(NNT):
                        nc.tensor.matmul(
                            ps[:],
                            vvs[h][:, nt, :],
                            pbt[:, nt, :, :],
                            start=(nt == 0),
                            stop=(nt == NNT - 1),
                        )
                    nc.vector.tensor_copy(
                        ctx_sb[h][:, jb * 512 : (jb + 1) * 512], ps[:]
                    )

                    def finish():
                        pass

                    return finish
                return pv

            def outproj_chain(tt, ec, store_eng=None):
                ps = ps_small.tile([P, 512], F32, tag="ps_small")
                for h in range(HPC):
                    nc.tensor.matmul(
                        ps[:],
                        ctx_sb[h][:, tt * P : (tt + 1) * P],
                        wo_sb[h][:, ec * 512 : (ec + 1) * 512],
                        start=(h == 0),
                        stop=(h == HPC - 1),
                    )
                ob = ob_pool.tile([P, 512], F32, tag="ob", name="ob")
                nc.vector.tensor_copy(ob[:], ps[:])
                (store_eng or nc.gpsimd).dma_start(
                    out[tt * P : (tt + 1) * P, ec * 512 : (ec + 1) * 512],
                    ob[:],
                )

            TAKES = {9: 1, 10: 1, 11: 1, 12: 1, 13: 1, 14: 1, 15: 2,
                     0: 2, 1: 2, 2: 2, 3: 2}
            TAKES_LAST = {0: 2, 1: 2, 2: 2, 3: 2,
                          8: 2, 9: 2, 10: 2, 11: 2, 12: 2, 13: 2, 14: 2, 15: 2}
            pv_pending = []
            fin2 = []
            op_pending = []
            op_next = []
            for jb in range(4):
                for h in range(HPC):
                    pbt = pbt_pool.tile([P, NNT, 4, P], BF16, tag="pbt",
                                        name="pbt")
                    for mtl in range(4):
                        unit_idx = 4 * h + mtl
                        scores_unit(h, jb, mtl, pbt)
                        if fin2:
                            fin2.pop(0)()
                        takes = TAKES_LAST if jb == 3 else TAKES
                        for _ in range(takes.get(unit_idx, 0)):
                            if op_pending:
                                outproj_chain(*op_pending.pop(0))
                    pv_pending.append(make_pv(h, jb, pbt))
                    if len(pv_pending) > 2:
                        fin2.append(pv_pending.pop(0)())
                    if h == 1 and op_next:
                        op_pending = op_next + op_pending
                        op_next = []
                op_next = [(4 * jb + i, ec) for i in range(4) for ec in range(4)]
            while pv_pending:
                fin2.append(pv_pending.pop(0)())
            while fin2:
                fin2.pop(0)()
            for chains in (op_pending, op_next):
                while chains:
                    outproj_chain(*chains.pop(0))


def _build(with_mask: bool):
    nc = bacc.Bacc("TRN2")
    xT = nc.dram_tensor("xT", [DIM, S], BF16, kind="ExternalInput")
    if with_mask:
        wqT = nc.dram_tensor("wqT", [DIM, DL], BF16, kind="ExternalInput")
        wkT = nc.dram_tensor("wkT", [DIM, DL], BF16, kind="ExternalInput")
        wvT = nc.dram_tensor("wvT", [DIM, DL], BF16, kind="ExternalInput")
        maskf = nc.dram_tensor("maskf", [S, S], F32, kind="ExternalInput")
        woT = nc.dram_tensor("woT", [DL, DIM], BF16, kind="ExternalInput")
        out = nc.dram_tensor("out", [S, DIM], F32, kind="ExternalOutput")
        with tile.TileContext(nc) as tc:
            _emit_masked(nc, tc, xT, wqT, wkT, wvT, woT, maskf, out)
    else:
        wq_arr = nc.dram_tensor("wq_arr", [HPC, P, NK, P], BF16,
                                kind="ExternalInput")
        wk_arr = nc.dram_tensor("wk_arr", [HPC, P, NK, P], BF16,
                                kind="ExternalInput")
        wv_arr = nc.dram_tensor("wv_arr", [HPC, P, NK, P], BF16,
                                kind="ExternalInput")
        woT = nc.dram_tensor("woT", [DL, DIM], BF16, kind="ExternalInput")
        out = nc.dram_tensor("out", [S, DIM], BF16, kind="ExternalOutput")
        with tile.TileContext(nc) as tc:
            _emit_fast(nc, tc, xT, wq_arr, wk_arr, wv_arr, woT, out)
    nc.finalize()
    return nc


def _get_program(with_mask: bool):
    if with_mask not in _PROGRAM_CACHE:
        _PROGRAM_CACHE[with_mask] = _build(with_mask)
    return _PROGRAM_CACHE[with_mask]


def _w_headwise(w, sl):
    """wq[sl,:].T as [HPC, P(part), NK, P] so each head's stationary tiles
    are one contiguous DMA."""
    bf = ml_dtypes.bfloat16
    A = np.asarray(w)[sl, :].T.astype(bf)          # [DIM, DL]
    A = A.reshape(NK, P, HPC, P).transpose(2, 1, 0, 3)
    return np.ascontiguousarray(A)


def _prep_in_maps(x, mask, wq, wk, wv, wo, with_mask):
    bf = ml_dtypes.bfloat16
    f32 = np.float32
    xTs = [np.ascontiguousarray(x[b].T.astype(bf)) for b in range(B)]
    if with_mask:
        maskf = np.ascontiguousarray(mask[0, 0].astype(f32) / SCALE)
    in_maps = []
    for c in range(N_CORES):
        b = c // HGROUPS
        g = c % HGROUPS
        sl = slice(g * DL, (g + 1) * DL)
        if with_mask:
            m = {
                "xT": xTs[b],
                "wqT": np.ascontiguousarray(wq[sl, :].T.astype(bf)),
                "wkT": np.ascontiguousarray(wk[sl, :].T.astype(bf)),
                "wvT": np.ascontiguousarray(wv[sl, :].T.astype(bf)),
                "woT": np.ascontiguousarray(wo[:, sl].T.astype(bf)),
                "maskf": maskf,
            }
        else:
            m = {
                "xT": xTs[b],
                "wq_arr": _w_headwise(wq, sl),
                "wk_arr": _w_headwise(wk, sl),
                "wv_arr": _w_headwise(wv, sl),
                "woT": np.ascontiguousarray(wo[:, sl].T.astype(bf)),
            }
        in_maps.append(m)
    return in_maps


def run_sharded(x, mask, wq, wk, wv, wo, trace=False, trace_kwargs=None):
    """Run the SPMD kernel; returns (full_output, BassKernelResults)."""
    with_mask = bool(np.any(np.asarray(mask)))
    nc = _get_program(with_mask)
    in_maps = _prep_in_maps(
        np.asarray(x), np.asarray(mask), np.asarray(wq), np.asarray(wk),
        np.asarray(wv), np.asarray(wo), with_mask,
    )
    kw = {}
    if trace:
        kw["trace"] = True
        if trace_kwargs:
            kw["trace_kwargs"] = trace_kwargs
    res = run_bass_kernel_spmd(nc, in_maps, list(range(N_CORES)), **kw)
    out = np.zeros((B, S, DIM), np.float32)
    for c in range(N_CORES):
        out[c // HGROUPS] += np.asarray(res.results[c]["out"],
                                        dtype=np.float32)
    return out, res


def kernel(**inputs):
    out, _ = run_sharded(
        inputs["x"], inputs["mask"], inputs["wq"], inputs["wk"], inputs["wv"],
        inputs["wo"],
    )
    return out


# revision 35
# speedup vs baseline: 1.0065x; 1.0065x over previous
"""Multi-head attention block (QKV proj -> softmax attention -> out proj) for
Trainium2, SPMD across 8 NeuronCores.

Sharding: batch (B=2) x head-groups (4 groups of 4 heads). Core c handles
batch c//4 and heads [4*(c%4), 4*(c%4)+4). Each core computes its partial
output contribution (context @ wo_slice.T); the host sums the 4 head-group
partials per batch (tensor-parallel row-sharded wo => the all-reduce is the
host-side gather).

Single head-granular software pipeline: the PE interleaves QKV projection
chains of head h+1 (and PV blocks of head h / h-1, out-proj at the end)
between the scores units of head h, so the scalar-exp / DVE-normalize /
x-bar-transpose softmax pipeline never stalls the PE. Softmax is standard
[query, key] orientation: scalar exp writes probs AND row sums via accum_out,
DVE normalizes in place, DMA x-bar transposes to [key, query] for PV.

PSUM (8 banks): scores 2x[128,1024], QKV/out-proj chains 3x[128,512],
PV 1x[128,512]. wo tiles alias 4 freed x tiles (same pool tags) to fit SBUF.
Partial outputs are stored bf16 and summed in f32 on the host.
"""

import sys

if "/opt/trn_rl_repo" not in sys.path:
    sys.path.insert(0, "/opt/trn_rl_repo")

from contextlib import ExitStack

import ml_dtypes
import numpy as np

import concourse.bacc as bacc
import concourse.tile as tile
from concourse import mybir
from concourse.bass_utils import run_bass_kernel_spmd

BF16 = mybir.dt.bfloat16
F32 = mybir.dt.float32

B, S, DIM = 2, 2048, 2048
HEADS, HD = 16, 128
P = 128
N_CORES = 8
HGROUPS = 4  # head groups (second shard axis is batch)
HPC = HEADS // HGROUPS  # heads per core = 4
DL = HPC * HD  # local head dims per core = 512
SCALE = 1.0 / float(np.sqrt(HD))

NK = DIM // P  # 16 contraction tiles for the projections
NMT = S // P  # 16 query tiles of 128 tokens
NNT = S // P  # 16 kv tiles of 128 tokens

_PROGRAM_CACHE = {}


def _emit_fast(nc, tc, xT, wq_arr, wk_arr, wv_arr, woT, out):
    """Pipelined no-mask variant."""
    with ExitStack() as octx:
        # ---- persistent SBUF pools ----
        xpool = octx.enter_context(tc.tile_pool(name="xt", bufs=1))
        xts = [xpool.tile([P, S], BF16, tag=f"x{kt}", name=f"x{kt}")
               for kt in range(NK)]
        wpools = {
            name: octx.enter_context(tc.tile_pool(name=f"w{name}", bufs=2))
            for name in ("q", "k", "v")
        }
        qpool = octx.enter_context(tc.tile_pool(name="qp", bufs=2))
        kpool = octx.enter_context(tc.tile_pool(name="kp", bufs=2))
        vTpool = octx.enter_context(tc.tile_pool(name="vTp", bufs=1))
        vvpool = octx.enter_context(tc.tile_pool(name="vvp", bufs=2))
        ctxpool = octx.enter_context(tc.tile_pool(name="ctxp", bufs=1))
        ctx_sb = [ctxpool.tile([P, S], BF16, tag=f"ctx{h}", name=f"ctx{h}")
                  for h in range(HPC)]
        pbm_pool = octx.enter_context(tc.tile_pool(name="pbm", bufs=5))
        pbt_pool = octx.enter_context(tc.tile_pool(name="pbt", bufs=3))
        stats = octx.enter_context(tc.tile_pool(name="stats", bufs=8))
        recp = octx.enter_context(tc.tile_pool(name="recp", bufs=16))
        ob_pool = octx.enter_context(tc.tile_pool(name="ob", bufs=6))

        # ---- PSUM pools: 4 + 3 + 1 banks ----
        ps_sc = octx.enter_context(
            tc.tile_pool(name="ps_sc", bufs=2, space="PSUM"))
        ps_qkv = octx.enter_context(
            tc.tile_pool(name="ps_qkv", bufs=3, space="PSUM"))
        ps_pv = octx.enter_context(
            tc.tile_pool(name="ps_pv", bufs=1, space="PSUM"))

        w_srcs = {"q": wq_arr, "k": wk_arr, "v": wv_arr}
        w_tiles = {}  # (name, h) -> tile
        q_sb = {}     # h -> [P, S] tile
        k_sb = {}
        vT_sb = {}
        vvs = {}
        pbts = {}     # (h, jb) -> tile
        wo_sb = {}    # h -> tile (aliases xts[h])

        def load_w(name, h):
            t = wpools[name].tile([P, NK, P], BF16, tag=f"w{name}",
                                  name=f"w{name}{h}")
            w_tiles[(name, h)] = t
            nc.gpsimd.dma_start(t[:], w_srcs[name][h])

        def load_wo(h):
            """wo tiles alias freed buffers: h<3 alias the w pools' spare
            buffer (freed after head-2 chains), h=3 aliases xts[0] (freed
            after the last slot-3 QKV chain)."""
            if h < 3:
                name = ("k", "q", "v")[h]
                t = wpools[name].tile([P, NK, P], BF16, tag=f"w{name}",
                                      name=f"wo{h}")
                src = woT[h * P: (h + 1) * P, :].rearrange(
                    "p (k c) -> p k c", k=NK)
            else:
                t = xpool.tile([P, S], BF16, tag="x0", name="wo3")
                src = woT[h * P: (h + 1) * P, :]
            wo_sb[h] = t
            nc.gpsimd.dma_start(t[:], src)

        def wo_slice(h, ec):
            t = wo_sb[h]
            if h < 3:
                return t[:, 4 * ec: 4 * (ec + 1), :]
            return t[:, ec * 512: (ec + 1) * 512]

        # ---- upfront DMA issue ----
        # ALL early loads go on gpsimd IN NEED ORDER: concurrent queues
        # share HBM bandwidth, so fanning out just slows the critical first
        # tiles -- a single in-order queue is the priority mechanism. The
        # prime's kt-major chains track the x stream tile by tile. Only
        # heads 0/1 load up front; head h+2's loads are emitted inside
        # slot h right after head h's chains so the pool-rotation WAR
        # waits never block the gpsimd queue.
        load_w("k", 0)
        nc.gpsimd.dma_start(xts[0][:], xT[0:P, :])
        load_w("q", 0)
        for kt in range(1, NK):
            nc.gpsimd.dma_start(xts[kt][:], xT[kt * P: (kt + 1) * P, :])
        load_w("v", 0)
        for name in ("k", "q", "v"):
            load_w(name, 1)

        # ---- work-item emitters ----
        def chain(name, h, quarter):
            """One QKV projection chain: 16 matmuls -> one 512-token quarter
            of q/k/vT for head h."""
            if name == "q":
                if h not in q_sb:
                    q_sb[h] = qpool.tile([P, S], BF16, tag="q", name=f"q{h}")
                dst = q_sb[h]
            elif name == "k":
                if h not in k_sb:
                    k_sb[h] = kpool.tile([P, S], BF16, tag="k", name=f"k{h}")
                dst = k_sb[h]
            else:
                if h not in vT_sb:
                    vT_sb[h] = vTpool.tile([P, S], BF16, tag="vT",
                                           name=f"vT{h}")
                dst = vT_sb[h]
            w = w_tiles[(name, h)]
            ps = ps_qkv.tile([P, 512], F32, tag="ps_qkv")
            for kt in range(NK):
                nc.tensor.matmul(
                    ps[:],
                    w[:, kt, :],
                    xts[kt][:, quarter * 512: (quarter + 1) * 512],
                    start=(kt == 0),
                    stop=(kt == NK - 1),
                )
            nc.vector.tensor_copy(
                dst[:, quarter * 512: (quarter + 1) * 512], ps[:])
            if name == "v" and quarter == 3:
                # whole vT tile ready -> transpose to [kv, d] for PV
                vvs[h] = vvpool.tile([P, NNT, P], BF16, tag="vv",
                                     name=f"vv{h}")
                nc.sync.dma_start(vvs[h][:], vT_sb[h][:], transpose=True)

        def scores_unit(h, u):
            """One 128-query scores unit: 4 matmuls + exp/accum + den/recip
            + in-place normalize + x-bar transpose into pbt[(h, u//4)]."""
            jb, mtl = u // 4, u % 4
            if (h, jb) not in pbts:
                pbts[(h, jb)] = pbt_pool.tile([P, NNT, 4, P], BF16,
                                              tag="pbt", name="pbt")
            pbt = pbts[(h, jb)]
            qt = q_sb[h][:, u * P: (u + 1) * P]
            pbm = pbm_pool.tile([P, S], BF16, tag="pbm", name="pbm")
            for c in range(2):
                ps = ps_sc.tile([P, 1024], F32, tag="ps_sc")
                for sub in range(2):
                    k0 = c * 1024 + sub * 512
                    nc.tensor.matmul(
                        ps[:, sub * 512: (sub + 1) * 512],
                        qt,
                        k_sb[h][:, k0: k0 + 512],
                        start=True,
                        stop=True,
                    )
                nc.scalar.activation(
                    pbm[:, c * 1024: (c + 1) * 1024],
                    ps[:],
                    mybir.ActivationFunctionType.Exp,
                    scale=SCALE,
                )
            # row sums on the DVE (keeps the scalar engine exp-only; the
            # accum_out + ACTIVATION_READ_ACCUMULATOR path cost 0.6us/unit
            # of scalar time, which paces slots 0 and 3)
            den = stats.tile([P, 1], F32, tag="den", name="den")
            nc.vector.tensor_reduce(
                den[:], pbm[:], mybir.AxisListType.X, mybir.AluOpType.add)
            rec = recp.tile([P, 1], F32, tag="rec", name="rec")
            nc.vector.reciprocal(rec[:], den[:])
            nc.vector.tensor_scalar_mul(pbm[:], pbm[:], rec[:])
            nc.sync.dma_start(pbt[:, :, mtl, :], pbm[:], transpose=True)

        def pv_block(h, jb, copy_eng=None):
            """ctx[h][:, jb*512:(jb+1)*512] = V_h.T @ probs block."""
            pbt = pbts.pop((h, jb))
            ps = ps_pv.tile([P, 512], F32, tag="ps_pv")
            for nt in range(NNT):
                nc.tensor.matmul(
                    ps[:],
                    vvs[h][:, nt, :],
                    pbt[:, nt, :, :],
                    start=(nt == 0),
                    stop=(nt == NNT - 1),
                )
            if copy_eng is None:
                nc.vector.tensor_copy(
                    ctx_sb[h][:, jb * 512: (jb + 1) * 512], ps[:])
            else:
                copy_eng.copy(
                    ctx_sb[h][:, jb * 512: (jb + 1) * 512], ps[:])

        op_count = [0]

        def outproj_chain(tt, ec):
            """out[tt, ec*512:...] partial: chain over the 4 heads."""
            i0 = op_count[0]
            if i0 >= 28 and i0 % 2 == 1:
                # tail: alternate into the scores psum banks (free once the
                # last exp has read them) to deepen the copy-WAR rotation
                pst = ps_sc.tile([P, 1024], F32, tag="ps_sc", name="ps_op")
                ps = pst[:, 0:512]
            else:
                ps = ps_qkv.tile([P, 512], F32, tag="ps_qkv")
            for h in range(HPC):
                nc.tensor.matmul(
                    ps[:],
                    ctx_sb[h][:, tt * P: (tt + 1) * P],
                    wo_slice(h, ec),
                    start=(h == 0),
                    stop=(h == HPC - 1),
                )
            i = op_count[0]
            op_count[0] += 1
            ob = ob_pool.tile([P, 512], BF16, tag="ob", name="ob")
            # The first 12 chains run inside slot 3 while the scalar engine
            # still streams exps -- keep its queue clear (DVE copy, gpsimd
            # store). In the tail, alternate copies DVE/scalar and stores
            # gpsimd/sync so no single queue paces the drain.
            # scalar joins the copy rotation only once the slot-3 exp
            # stream (which shares its queue) has surely drained
            if i < 28 or i % 2 == 0:
                nc.vector.tensor_copy(ob[:], ps[:])
            else:
                nc.scalar.copy(ob[:], ps[:])
            if i < 12:
                store_eng = nc.gpsimd
            else:
                store_eng = nc.gpsimd if i % 2 == 0 else nc.sync
            store_eng.dma_start(
                out[tt * P: (tt + 1) * P, ec * 512: (ec + 1) * 512], ob[:])

        # ---- schedule ----
        # Prime: k0..k3, q0, q1 of head 0 emitted kt-major as six concurrent
        # accumulation chains (3 on ps_qkv banks, 3 on the two ps_sc tiles'
        # 512-halves) so the PE tracks the x DMA stream tile-by-tile instead
        # of idling until the whole 8MB lands.
        q_sb[0] = qpool.tile([P, S], BF16, tag="q", name="q0")
        k_sb[0] = kpool.tile([P, S], BF16, tag="k", name="k0")
        pr_qkv = [ps_qkv.tile([P, 512], F32, tag="ps_qkv", name=f"prq{i}")
                  for i in range(3)]
        pr_sc = [ps_sc.tile([P, 1024], F32, tag="ps_sc", name=f"prs{i}")
                 for i in range(2)]
        # (dst, quarter, psum AP) -- k chains first: round 0's k matmuls
        # need only wk0 + x0, covering the wq0 transfer
        prime = [
            (k_sb[0], 0, pr_qkv[0][:]),
            (k_sb[0], 1, pr_qkv[1][:]),
            (k_sb[0], 2, pr_sc[0][:, 0:512]),
            (k_sb[0], 3, pr_sc[0][:, 512:1024]),
            (q_sb[0], 0, pr_qkv[2][:]),
            (q_sb[0], 1, pr_sc[1][:, 0:512]),
        ]
        wk0 = w_tiles[("k", 0)]
        wq0 = w_tiles[("q", 0)]
        for kt in range(NK):
            for dst, quarter, ps in prime:
                nc.tensor.matmul(
                    ps,
                    (wq0 if dst is q_sb[0] else wk0)[:, kt, :],
                    xts[kt][:, quarter * 512: (quarter + 1) * 512],
                    start=(kt == 0),
                    stop=(kt == NK - 1),
                )
        for dst, quarter, ps in prime:
            nc.vector.tensor_copy(
                dst[:, quarter * 512: (quarter + 1) * 512], ps)

        # Fillers per slot h, keyed by unit index. Steady-state slots carry:
        # rest-of-h chains (q1..q3, v0..v3), next head's k0..k3 + q0, PV of
        # (h-1, jb2/jb3) and (h, jb0/jb1). Head h+2's weight DMAs are
        # re-emitted just after head h's chains so their pool-rotation waits
        # resolve instantly. Slot 3 replaces next-head chains with out-proj
        # of jb0 and pulls its own v chains earlier so vv(h3) beats
        # PV(h3, jb0).
        op_queue = [(tt, ec) for tt in range(NMT) for ec in range(4)]

        def take_op(n):
            for _ in range(n):
                if op_queue:
                    outproj_chain(*op_queue.pop(0))

        for h in range(HPC):
            last = h == HPC - 1
            if not last:
                fillers = {
                    0: [("loadw", "k", h + 2), ("chain", "q", h, 1)],
                    1: [("chain", "v", h, 0)],
                    2: [("pv", h - 1, 2)],
                    3: [("chain", "v", h, 1)],
                    4: [("chain", "v", h, 2)],
                    5: [("pv", h - 1, 3)],
                    6: [("chain", "q", h, 2)],
                    7: [("chain", "v", h, 3)],
                    8: [("chain", "q", h, 3), ("loadw", "v", h + 2)],
                    9: [("chain", "k", h + 1, 0), ("loadw", "q", h + 2)],
                    10: [("chain", "k", h + 1, 1)],
                    11: [("chain", "k", h + 1, 2)],
                    12: [("pv", h, 0)],
                    13: [("chain", "k", h + 1, 3)],
                    14: [("pv", h, 1)],
                    15: [("chain", "q", h + 1, 0)],
                }
                if h == 0:
                    del fillers[0][1]  # q1(h0) ran in the prime
                if h == 2:
                    # wo0..wo2 alias w-pool buffers freed by head-2's chains
                    fillers[0].append(("loadwo", 0))
                    fillers[8].append(("loadwo", 2))
                    fillers[9].append(("loadwo", 1))
            else:
                # v chains front-loaded so the vv transpose queues ahead of
                # most pbm transposes and beats PV(h3, 0) at u10
                fillers = {
                    0: [("chain", "q", h, 1)],
                    1: [("chain", "v", h, 0)],
                    2: [("chain", "v", h, 1)],
                    3: [("pv", h - 1, 2)],
                    4: [("chain", "v", h, 2)],
                    5: [("chain", "v", h, 3)],
                    6: [("chain", "q", h, 2)],
                    7: [("pv", h - 1, 3)],
                    8: [("chain", "q", h, 3), ("loadwo", 3)],
                    10: [("pv", h, 0)],
                    11: [("op", 3)],
                    12: [("op", 3)],
                    13: [("pv", h, 1)],
                    14: [("op", 3)],
                    15: [("op", 3)],
                }
            for u in range(16):
                scores_unit(h, u)
                for item in fillers.get(u, []):
                    if item[0] == "chain":
                        chain(item[1], item[2], item[3])
                    elif item[0] == "pv":
                        if item[1] >= 0:
                            pv_block(item[1], item[2])
                    elif item[0] == "loadw":
                        if item[2] < HPC:
                            load_w(item[1], item[2])
                    elif item[0] == "loadwo":
                        load_wo(item[1])
                    else:
                        take_op(item[1])

        # ---- tail ----
        # Ready op chains go BEFORE each PV block: the PE queue is in-order,
        # so a PV waiting on slot-3's exp-paced transposes must not block
        # chains whose inputs are already complete.
        take_op(16)
        pv_block(3, 2, copy_eng=nc.scalar)
        take_op(16)
        pv_block(3, 3, copy_eng=nc.scalar)
        take_op(len(op_queue))


def _emit_masked(nc, tc, xT, wqT, wkT, wvT, woT, maskf, out):
    """Two-phase variant with additive mask (baseline path; mask is zero in
    the benchmark so this is correctness-only)."""
    with ExitStack() as octx:
        planes = octx.enter_context(tc.tile_pool(name="planes", bufs=1))
        q_sb = [planes.tile([P, S], BF16, tag=f"q{h}", name=f"q{h}") for h in range(HPC)]
        k_sb = [planes.tile([P, S], BF16, tag=f"k{h}", name=f"k{h}") for h in range(HPC)]
        ctx_sb = [planes.tile([P, S], BF16, tag=f"ctx{h}", name=f"ctx{h}")
                  for h in range(HPC)]
        vv_pool = octx.enter_context(tc.tile_pool(name="vv", bufs=1))
        vvs = [vv_pool.tile([P, NNT, P], BF16, tag=f"vv{h}", name=f"vv{h}")
               for h in range(HPC)]

        ps_sc = octx.enter_context(tc.tile_pool(name="ps_sc", bufs=3, space="PSUM"))
        ps_small = octx.enter_context(
            tc.tile_pool(name="ps_small", bufs=2, space="PSUM")
        )

        with ExitStack() as ctx:
            wpool = ctx.enter_context(tc.tile_pool(name="wqkv", bufs=1))
            w_sb = {}
            for name, src in (("q", wqT), ("k", wkT), ("v", wvT)):
                w_sb[name] = wpool.tile([P, NK, DL], BF16, tag=f"w{name}",
                                        name=f"w{name}")
            vT_sb = [wpool.tile([P, S], BF16, tag=f"vt{h}", name=f"vt{h}")
                     for h in range(HPC)]
            xpool = ctx.enter_context(tc.tile_pool(name="xt", bufs=1))
            xts = [xpool.tile([P, S], BF16, tag=f"x{kt}", name=f"x{kt}")
                   for kt in range(NK)]
            for kt in range(NK):
                nc.sync.dma_start(xts[kt][:], xT[kt * P : (kt + 1) * P, :])
            for name, src in (("q", wqT), ("k", wkT), ("v", wvT)):
                for kt in range(NK):
                    nc.gpsimd.dma_start(
                        w_sb[name][:, kt, :], src[kt * P : (kt + 1) * P, :]
                    )

            for h in range(HPC):
                for name, dsts in (("q", q_sb), ("k", k_sb), ("v", vT_sb)):
                    for half in range(2):
                        ps = ps_sc.tile([P, 1024], F32, tag="ps_sc")
                        for kt in range(NK):
                            for mc in range(2):
                                m0 = half * 1024 + mc * 512
                                nc.tensor.matmul(
                                    ps[:, mc * 512 : (mc + 1) * 512],
                                    w_sb[name][:, kt, h * P : (h + 1) * P],
                                    xts[kt][:, m0 : m0 + 512],
                                    start=(kt == 0),
                                    stop=(kt == NK - 1),
                                )
                        nc.any.tensor_copy(
                            dsts[h][:, half * 1024 : (half + 1) * 1024], ps[:]
                        )
                nc.sync.dma_start(vvs[h][:], vT_sb[h][:], transpose=True)

        with ExitStack() as ctx:
            wopool = ctx.enter_context(tc.tile_pool(name="wo", bufs=1))
            wo_sb = [wopool.tile([P, DIM], BF16, tag=f"wo{h}", name=f"wo{h}")
                     for h in range(HPC)]
            for h in range(HPC):
                nc.gpsimd.dma_start(wo_sb[h][:], woT[h * P : (h + 1) * P, :])

            pbm_pool = ctx.enter_context(tc.tile_pool(name="pbm", bufs=14))
            pbt_pool = ctx.enter_context(tc.tile_pool(name="pbt", bufs=4))
            stats = ctx.enter_context(tc.tile_pool(name="stats", bufs=8))
            recp = ctx.enter_context(tc.tile_pool(name="recp", bufs=16))
            ob_pool = ctx.enter_context(tc.tile_pool(name="ob", bufs=3))
            mpool = ctx.enter_context(tc.tile_pool(name="mask", bufs=3))

            def scores_unit(h, jb, mtl, pbt_dst):
                mt = 4 * jb + mtl
                qt = q_sb[h][:, mt * P : (mt + 1) * P]
                pbm = pbm_pool.tile([P, S], BF16, tag="pbm", name="pbm")
                accs = stats.tile([P, 2], F32, tag="accs", name="accs")
                mts = mpool.tile([P, S], F32, tag="mt", name="mts")
                nc.gpsimd.dma_start(mts[:], maskf[mt * P : (mt + 1) * P, :])
                for c in range(2):
                    ps = ps_sc.tile([P, 1024], F32, tag="ps_sc")
                    for sub in range(2):
                        k0 = c * 1024 + sub * 512
                        nc.tensor.matmul(
                            ps[:, sub * 512 : (sub + 1) * 512],
                            qt,
                            k_sb[h][:, k0 : k0 + 512],
                            start=True,
                            stop=True,
                        )
                    nc.vector.tensor_add(
                        ps[:], ps[:], mts[:, c * 1024 : (c + 1) * 1024]
                    )
                    nc.scalar.activation(
                        pbm[:, c * 1024 : (c + 1) * 1024],
                        ps[:],
                        mybir.ActivationFunctionType.Exp,
                        scale=SCALE,
                        accum_out=accs[:, c : c + 1],
                    )
                den = stats.tile([P, 1], F32, tag="den", name="den")
                nc.vector.tensor_add(den[:], accs[:, 0:1], accs[:, 1:2])
                rec = recp.tile([P, 1], F32, tag="rec", name="rec")
                nc.vector.reciprocal(rec[:], den[:])
                nc.vector.tensor_scalar_mul(pbm[:], pbm[:], rec[:])
                nc.sync.dma_start(
                    pbt_dst[:, :, mtl, :], pbm[:], transpose=True
                )

            def make_pv(h, jb, pbt):
                def pv():
                    ps = ps_small.tile([P, 512], F32, tag="ps_small")
                    for nt in range(NNT):
                        nc.tensor.matmul(
                            ps[:],
                            vvs[h][:, nt, :],
                            pbt[:, nt, :, :],
                            start=(nt == 0),
                            stop=(nt == NNT - 1),
                        )
                    nc.vector.tensor_copy(
                        ctx_sb[h][:, jb * 512 : (jb + 1) * 512], ps[:]
                    )

                    def finish():
                        pass

                    return finish
                return pv

            def outproj_chain(tt, ec, store_eng=None):
                ps = ps_small.tile([P, 512], F32, tag="ps_small")
                for h in range(HPC):
                    nc.tensor.matmul(
                        ps[:],
                        ctx_sb[h][:, tt * P : (tt + 1) * P],
                        wo_sb[h][:, ec * 512 : (ec + 1) * 512],
                        start=(h == 0),
                        stop=(h == HPC - 1),
                    )
                ob = ob_pool.tile([P, 512], F32, tag="ob", name="ob")
                nc.vector.tensor_copy(ob[:], ps[:])
                (store_eng or nc.gpsimd).dma_start(
                    out[tt * P : (tt + 1) * P, ec * 512 : (ec + 1) * 512],
                    ob[:],
                )

            TAKES = {9: 1, 10: 1, 11: 1, 12: 1, 13: 1, 14: 1, 15: 2,
                     0: 2, 1: 2, 2: 2, 3: 2}
            TAKES_LAST = {0: 2, 1: 2, 2: 2, 3: 2,
                          8: 2, 9: 2, 10: 2, 11: 2, 12: 2, 13: 2, 14: 2, 15: 2}
            pv_pending = []
            fin2 = []
            op_pending = []
            op_next = []
            for jb in range(4):
                for h in range(HPC):
                    pbt = pbt_pool.tile([P, NNT, 4, P], BF16, tag="pbt",
                                        name="pbt")
                    for mtl in range(4):
                        unit_idx = 4 * h + mtl
                        scores_unit(h, jb, mtl, pbt)
                        if fin2:
                            fin2.pop(0)()
                        takes = TAKES_LAST if jb == 3 else TAKES
                        for _ in range(takes.get(unit_idx, 0)):
                            if op_pending:
                                outproj_chain(*op_pending.pop(0))
                    pv_pending.append(make_pv(h, jb, pbt))
                    if len(pv_pending) > 2:
                        fin2.append(pv_pending.pop(0)())
                    if h == 1 and op_next:
                        op_pending = op_next + op_pending
                        op_next = []
                op_next = [(4 * jb + i, ec) for i in range(4) for ec in range(4)]
            while pv_pending:
                fin2.append(pv_pending.pop(0)())
            while fin2:
                fin2.pop(0)()
            for chains in (op_pending, op_next):
                while chains:
                    outproj_chain(*chains.pop(0))


def _build(with_mask: bool):
    nc = bacc.Bacc("TRN2")
    xT = nc.dram_tensor("xT", [DIM, S], BF16, kind="ExternalInput")
    if with_mask:
        wqT = nc.dram_tensor("wqT", [DIM, DL], BF16, kind="ExternalInput")
        wkT = nc.dram_tensor("wkT", [DIM, DL], BF16, kind="ExternalInput")
        wvT = nc.dram_tensor("wvT", [DIM, DL], BF16, kind="ExternalInput")
        maskf = nc.dram_tensor("maskf", [S, S], F32, kind="ExternalInput")
        woT = nc.dram_tensor("woT", [DL, DIM], BF16, kind="ExternalInput")
        out = nc.dram_tensor("out", [S, DIM], F32, kind="ExternalOutput")
        with tile.TileContext(nc) as tc:
            _emit_masked(nc, tc, xT, wqT, wkT, wvT, woT, maskf, out)
    else:
        wq_arr = nc.dram_tensor("wq_arr", [HPC, P, NK, P], BF16,
                                kind="ExternalInput")
        wk_arr = nc.dram_tensor("wk_arr", [HPC, P, NK, P], BF16,
                                kind="ExternalInput")
        wv_arr = nc.dram_tensor("wv_arr", [HPC, P, NK, P], BF16,
                                kind="ExternalInput")
        woT = nc.dram_tensor("woT", [DL, DIM], BF16, kind="ExternalInput")
        out = nc.dram_tensor("out", [S, DIM], BF16, kind="ExternalOutput")
        with tile.TileContext(nc) as tc:
            _emit_fast(nc, tc, xT, wq_arr, wk_arr, wv_arr, woT, out)
    nc.finalize()
    return nc


def _get_program(with_mask: bool):
    if with_mask not in _PROGRAM_CACHE:
        _PROGRAM_CACHE[with_mask] = _build(with_mask)
    return _PROGRAM_CACHE[with_mask]


def _w_headwise(w, sl):
    """wq[sl,:].T as [HPC, P(part), NK, P] so each head's stationary tiles
    are one contiguous DMA."""
    bf = ml_dtypes.bfloat16
    A = np.asarray(w)[sl, :].T.astype(bf)          # [DIM, DL]
    A = A.reshape(NK, P, HPC, P).transpose(2, 1, 0, 3)
    return np.ascontiguousarray(A)


def _prep_in_maps(x, mask, wq, wk, wv, wo, with_mask):
    bf = ml_dtypes.bfloat16
    f32 = np.float32
    xTs = [np.ascontiguousarray(x[b].T.astype(bf)) for b in range(B)]
    if with_mask:
        maskf = np.ascontiguousarray(mask[0, 0].astype(f32) / SCALE)
    in_maps = []
    for c in range(N_CORES):
        b = c // HGROUPS
        g = c % HGROUPS
        sl = slice(g * DL, (g + 1) * DL)
        if with_mask:
            m = {
                "xT": xTs[b],
                "wqT": np.ascontiguousarray(wq[sl, :].T.astype(bf)),
                "wkT": np.ascontiguousarray(wk[sl, :].T.astype(bf)),
                "wvT": np.ascontiguousarray(wv[sl, :].T.astype(bf)),
                "woT": np.ascontiguousarray(wo[:, sl].T.astype(bf)),
                "maskf": maskf,
            }
        else:
            m = {
                "xT": xTs[b],
                "wq_arr": _w_headwise(wq, sl),
                "wk_arr": _w_headwise(wk, sl),
                "wv_arr": _w_headwise(wv, sl),
                "woT": np.ascontiguousarray(wo[:, sl].T.astype(bf)),
            }
        in_maps.append(m)
    return in_maps


def run_sharded(x, mask, wq, wk, wv, wo, trace=False, trace_kwargs=None):
    """Run the SPMD kernel; returns (full_output, BassKernelResults)."""
    with_mask = bool(np.any(np.asarray(mask)))
    nc = _get_program(with_mask)
    in_maps = _prep_in_maps(
        np.asarray(x), np.asarray(mask), np.asarray(wq), np.asarray(wk),
        np.asarray(wv), np.asarray(wo), with_mask,
    )
    kw = {}
    if trace:
        kw["trace"] = True
        if trace_kwargs:
            kw["trace_kwargs"] = trace_kwargs
    res = run_bass_kernel_spmd(nc, in_maps, list(range(N_CORES)), **kw)
    out = np.zeros((B, S, DIM), np.float32)
    for c in range(N_CORES):
        out[c // HGROUPS] += np.asarray(res.results[c]["out"],
                                        dtype=np.float32)
    return out, res


def kernel(**inputs):
    out, _ = run_sharded(
        inputs["x"], inputs["mask"], inputs["wq"], inputs["wk"], inputs["wv"],
        inputs["wo"],
    )
    return out


# revision 37
# speedup vs baseline: 1.0158x; 1.0092x over previous
"""Multi-head attention block (QKV proj -> softmax attention -> out proj) for
Trainium2, SPMD across 8 NeuronCores.

Sharding: batch (B=2) x head-groups (4 groups of 4 heads). Core c handles
batch c//4 and heads [4*(c%4), 4*(c%4)+4). Each core computes its partial
output contribution (context @ wo_slice.T); the host sums the 4 head-group
partials per batch (tensor-parallel row-sharded wo => the all-reduce is the
host-side gather).

Single head-granular software pipeline: the PE interleaves QKV projection
chains of head h+1 (and PV blocks of head h / h-1, out-proj at the end)
between the scores units of head h, so the scalar-exp / DVE-normalize /
x-bar-transpose softmax pipeline never stalls the PE. Softmax is standard
[query, key] orientation: scalar exp writes probs AND row sums via accum_out,
DVE normalizes in place, DMA x-bar transposes to [key, query] for PV.

PSUM (8 banks): scores 2x[128,1024], QKV/out-proj chains 3x[128,512],
PV 1x[128,512]. wo tiles alias 4 freed x tiles (same pool tags) to fit SBUF.
Partial outputs are stored bf16 and summed in f32 on the host.
"""

import sys

if "/opt/trn_rl_repo" not in sys.path:
    sys.path.insert(0, "/opt/trn_rl_repo")

from contextlib import ExitStack

import ml_dtypes
import numpy as np

import concourse.bacc as bacc
import concourse.tile as tile
from concourse import mybir
from concourse.bass_utils import run_bass_kernel_spmd

BF16 = mybir.dt.bfloat16
F32 = mybir.dt.float32

B, S, DIM = 2, 2048, 2048
HEADS, HD = 16, 128
P = 128
N_CORES = 8
HGROUPS = 4  # head groups (second shard axis is batch)
HPC = HEADS // HGROUPS  # heads per core = 4
DL = HPC * HD  # local head dims per core = 512
SCALE = 1.0 / float(np.sqrt(HD))

NK = DIM // P  # 16 contraction tiles for the projections
NMT = S // P  # 16 query tiles of 128 tokens
NNT = S // P  # 16 kv tiles of 128 tokens

_PROGRAM_CACHE = {}


def _emit_fast(nc, tc, xT, wq_arr, wk_arr, wv_arr, woT, out):
    """Pipelined no-mask variant."""
    with ExitStack() as octx:
        # ---- persistent SBUF pools ----
        xpool = octx.enter_context(tc.tile_pool(name="xt", bufs=1))
        xts = [xpool.tile([P, S], BF16, tag=f"x{kt}", name=f"x{kt}")
               for kt in range(NK)]
        wpools = {
            name: octx.enter_context(tc.tile_pool(name=f"w{name}", bufs=2))
            for name in ("q", "k", "v")
        }
        qpool = octx.enter_context(tc.tile_pool(name="qp", bufs=2))
        kpool = octx.enter_context(tc.tile_pool(name="kp", bufs=2))
        vTpool = octx.enter_context(tc.tile_pool(name="vTp", bufs=1))
        vvpool = octx.enter_context(tc.tile_pool(name="vvp", bufs=2))
        ctxpool = octx.enter_context(tc.tile_pool(name="ctxp", bufs=1))
        ctx_sb = [ctxpool.tile([P, S], BF16, tag=f"ctx{h}", name=f"ctx{h}")
                  for h in range(HPC)]
        pbm_pool = octx.enter_context(tc.tile_pool(name="pbm", bufs=5))
        pbt_pool = octx.enter_context(tc.tile_pool(name="pbt", bufs=3))
        stats = octx.enter_context(tc.tile_pool(name="stats", bufs=8))
        recp = octx.enter_context(tc.tile_pool(name="recp", bufs=16))
        ob_pool = octx.enter_context(tc.tile_pool(name="ob", bufs=6))

        # ---- PSUM pools: 4 + 3 + 1 banks ----
        ps_sc = octx.enter_context(
            tc.tile_pool(name="ps_sc", bufs=2, space="PSUM"))
        ps_qkv = octx.enter_context(
            tc.tile_pool(name="ps_qkv", bufs=3, space="PSUM"))
        ps_pv = octx.enter_context(
            tc.tile_pool(name="ps_pv", bufs=1, space="PSUM"))

        w_srcs = {"q": wq_arr, "k": wk_arr, "v": wv_arr}
        w_tiles = {}  # (name, h) -> tile
        q_sb = {}     # h -> [P, S] tile
        k_sb = {}
        vT_sb = {}
        vvs = {}
        pbts = {}     # (h, jb) -> tile
        wo_sb = {}    # h -> tile (aliases xts[h])

        def load_w(name, h):
            t = wpools[name].tile([P, NK, P], BF16, tag=f"w{name}",
                                  name=f"w{name}{h}")
            w_tiles[(name, h)] = t
            nc.gpsimd.dma_start(t[:], w_srcs[name][h])

        def load_wo(h):
            """wo tiles alias freed buffers: h<3 alias the w pools' spare
            buffer (freed after head-2 chains), h=3 aliases xts[0] (freed
            after the last slot-3 QKV chain)."""
            if h < 3:
                name = ("k", "q", "v")[h]
                t = wpools[name].tile([P, NK, P], BF16, tag=f"w{name}",
                                      name=f"wo{h}")
                src = woT[h * P: (h + 1) * P, :].rearrange(
                    "p (k c) -> p k c", k=NK)
            else:
                t = xpool.tile([P, S], BF16, tag="x0", name="wo3")
                src = woT[h * P: (h + 1) * P, :]
            wo_sb[h] = t
            nc.gpsimd.dma_start(t[:], src)

        def wo_slice(h, ec):
            t = wo_sb[h]
            if h < 3:
                return t[:, 4 * ec: 4 * (ec + 1), :]
            return t[:, ec * 512: (ec + 1) * 512]

        # ---- upfront DMA issue ----
        # ALL early loads go on gpsimd IN NEED ORDER: concurrent queues
        # share HBM bandwidth, so fanning out just slows the critical first
        # tiles -- a single in-order queue is the priority mechanism. The
        # prime's kt-major chains track the x stream tile by tile. Only
        # heads 0/1 load up front; head h+2's loads are emitted inside
        # slot h right after head h's chains so the pool-rotation WAR
        # waits never block the gpsimd queue.
        load_w("k", 0)
        nc.gpsimd.dma_start(xts[0][:], xT[0:P, :])
        load_w("q", 0)
        for kt in range(1, NK):
            nc.gpsimd.dma_start(xts[kt][:], xT[kt * P: (kt + 1) * P, :])
        load_w("v", 0)
        for name in ("k", "q", "v"):
            load_w(name, 1)

        # ---- work-item emitters ----
        def chain(name, h, quarter):
            """One QKV projection chain: 16 matmuls -> one 512-token quarter
            of q/k/vT for head h."""
            if name == "q":
                if h not in q_sb:
                    q_sb[h] = qpool.tile([P, S], BF16, tag="q", name=f"q{h}")
                dst = q_sb[h]
            elif name == "k":
                if h not in k_sb:
                    k_sb[h] = kpool.tile([P, S], BF16, tag="k", name=f"k{h}")
                dst = k_sb[h]
            else:
                if h not in vT_sb:
                    vT_sb[h] = vTpool.tile([P, S], BF16, tag="vT",
                                           name=f"vT{h}")
                dst = vT_sb[h]
            w = w_tiles[(name, h)]
            ps = ps_qkv.tile([P, 512], F32, tag="ps_qkv")
            for kt in range(NK):
                nc.tensor.matmul(
                    ps[:],
                    w[:, kt, :],
                    xts[kt][:, quarter * 512: (quarter + 1) * 512],
                    start=(kt == 0),
                    stop=(kt == NK - 1),
                )
            nc.vector.tensor_copy(
                dst[:, quarter * 512: (quarter + 1) * 512], ps[:])
            if name == "v" and quarter == 3:
                # whole vT tile ready -> transpose to [kv, d] for PV
                vvs[h] = vvpool.tile([P, NNT, P], BF16, tag="vv",
                                     name=f"vv{h}")
                nc.sync.dma_start(vvs[h][:], vT_sb[h][:], transpose=True)

        def scores_unit(h, u):
            """One 128-query scores unit: 4 matmuls + exp/accum + den/recip
            + in-place normalize + x-bar transpose into pbt[(h, u//4)]."""
            jb, mtl = u // 4, u % 4
            if (h, jb) not in pbts:
                pbts[(h, jb)] = pbt_pool.tile([P, NNT, 4, P], BF16,
                                              tag="pbt", name="pbt")
            pbt = pbts[(h, jb)]
            qt = q_sb[h][:, u * P: (u + 1) * P]
            pbm = pbm_pool.tile([P, S], BF16, tag="pbm", name="pbm")
            for c in range(2):
                ps = ps_sc.tile([P, 1024], F32, tag="ps_sc")
                for sub in range(2):
                    k0 = c * 1024 + sub * 512
                    nc.tensor.matmul(
                        ps[:, sub * 512: (sub + 1) * 512],
                        qt,
                        k_sb[h][:, k0: k0 + 512],
                        start=True,
                        stop=True,
                    )
                nc.scalar.activation(
                    pbm[:, c * 1024: (c + 1) * 1024],
                    ps[:],
                    mybir.ActivationFunctionType.Exp,
                    scale=SCALE,
                )
            # row sums on the DVE (keeps the scalar engine exp-only; the
            # accum_out + ACTIVATION_READ_ACCUMULATOR path cost 0.6us/unit
            # of scalar time, which paces slots 0 and 3)
            den = stats.tile([P, 1], F32, tag="den", name="den")
            nc.vector.tensor_reduce(
                den[:], pbm[:], mybir.AxisListType.X, mybir.AluOpType.add)
            rec = recp.tile([P, 1], F32, tag="rec", name="rec")
            nc.vector.reciprocal(rec[:], den[:])
            nc.vector.tensor_scalar_mul(pbm[:], pbm[:], rec[:])
            nc.sync.dma_start(pbt[:, :, mtl, :], pbm[:], transpose=True)

        def pv_block(h, jb, copy_eng=None):
            """ctx[h][:, jb*512:(jb+1)*512] = V_h.T @ probs block."""
            pbt = pbts.pop((h, jb))
            ps = ps_pv.tile([P, 512], F32, tag="ps_pv")
            for nt in range(NNT):
                nc.tensor.matmul(
                    ps[:],
                    vvs[h][:, nt, :],
                    pbt[:, nt, :, :],
                    start=(nt == 0),
                    stop=(nt == NNT - 1),
                )
            if copy_eng is None:
                nc.vector.tensor_copy(
                    ctx_sb[h][:, jb * 512: (jb + 1) * 512], ps[:])
            else:
                copy_eng.copy(
                    ctx_sb[h][:, jb * 512: (jb + 1) * 512], ps[:])

        op_count = [0]

        def outproj_chain(tt, ec):
            """out[tt, ec*512:...] partial: chain over the 4 heads."""
            i0 = op_count[0]
            if i0 >= 28 and i0 % 2 == 1:
                # tail: alternate into the scores psum banks (free once the
                # last exp has read them) to deepen the copy-WAR rotation
                pst = ps_sc.tile([P, 1024], F32, tag="ps_sc", name="ps_op")
                ps = pst[:, 0:512]
            else:
                ps = ps_qkv.tile([P, 512], F32, tag="ps_qkv")
            for h in range(HPC):
                nc.tensor.matmul(
                    ps[:],
                    ctx_sb[h][:, tt * P: (tt + 1) * P],
                    wo_slice(h, ec),
                    start=(h == 0),
                    stop=(h == HPC - 1),
                )
            i = op_count[0]
            op_count[0] += 1
            ob = ob_pool.tile([P, 512], BF16, tag="ob", name="ob")
            # The first 12 chains run inside slot 3 while the scalar engine
            # still streams exps -- keep its queue clear (DVE copy, gpsimd
            # store). In the tail, alternate copies DVE/scalar and stores
            # gpsimd/sync so no single queue paces the drain.
            # Tail engine split (GPSIMD cannot read PSUM, so copies stay on
            # DVE/scalar): slot-3 chains keep the scalar queue clear for the
            # exp stream; tail chains alternate both copy and store engines.
            if i < 12 or i % 2 == 0:
                nc.vector.tensor_copy(ob[:], ps[:])
            else:
                nc.scalar.copy(ob[:], ps[:])
            if i < 12:
                store_eng = nc.gpsimd
            else:
                store_eng = nc.gpsimd if i % 2 == 0 else nc.sync
            store_eng.dma_start(
                out[tt * P: (tt + 1) * P, ec * 512: (ec + 1) * 512], ob[:])

        # ---- schedule ----
        # Prime: k0..k3, q0, q1 of head 0 emitted kt-major as six concurrent
        # accumulation chains (3 on ps_qkv banks, 3 on the two ps_sc tiles'
        # 512-halves) so the PE tracks the x DMA stream tile-by-tile instead
        # of idling until the whole 8MB lands.
        q_sb[0] = qpool.tile([P, S], BF16, tag="q", name="q0")
        k_sb[0] = kpool.tile([P, S], BF16, tag="k", name="k0")
        pr_qkv = [ps_qkv.tile([P, 512], F32, tag="ps_qkv", name=f"prq{i}")
                  for i in range(3)]
        pr_sc = [ps_sc.tile([P, 1024], F32, tag="ps_sc", name=f"prs{i}")
                 for i in range(2)]
        # (dst, quarter, psum AP) -- k chains first: round 0's k matmuls
        # need only wk0 + x0, covering the wq0 transfer
        prime = [
            (k_sb[0], 0, pr_qkv[0][:]),
            (k_sb[0], 1, pr_qkv[1][:]),
            (k_sb[0], 2, pr_sc[0][:, 0:512]),
            (k_sb[0], 3, pr_sc[0][:, 512:1024]),
            (q_sb[0], 0, pr_qkv[2][:]),
            (q_sb[0], 1, pr_sc[1][:, 0:512]),
        ]
        wk0 = w_tiles[("k", 0)]
        wq0 = w_tiles[("q", 0)]
        for kt in range(NK):
            for dst, quarter, ps in prime:
                nc.tensor.matmul(
                    ps,
                    (wq0 if dst is q_sb[0] else wk0)[:, kt, :],
                    xts[kt][:, quarter * 512: (quarter + 1) * 512],
                    start=(kt == 0),
                    stop=(kt == NK - 1),
                )
        for dst, quarter, ps in prime:
            nc.vector.tensor_copy(
                dst[:, quarter * 512: (quarter + 1) * 512], ps)

        # Fillers per slot h, keyed by unit index. Steady-state slots carry:
        # rest-of-h chains (q1..q3, v0..v3), next head's k0..k3 + q0, PV of
        # (h-1, jb2/jb3) and (h, jb0/jb1). Head h+2's weight DMAs are
        # re-emitted just after head h's chains so their pool-rotation waits
        # resolve instantly. Slot 3 replaces next-head chains with out-proj
        # of jb0 and pulls its own v chains earlier so vv(h3) beats
        # PV(h3, jb0).
        op_queue = [(tt, ec) for tt in range(NMT) for ec in range(4)]

        def take_op(n):
            for _ in range(n):
                if op_queue:
                    outproj_chain(*op_queue.pop(0))

        for h in range(HPC):
            last = h == HPC - 1
            if not last:
                fillers = {
                    0: [("loadw", "k", h + 2), ("chain", "q", h, 1)],
                    1: [("chain", "v", h, 0)],
                    2: [("pv", h - 1, 2)],
                    3: [("chain", "v", h, 1)],
                    4: [("chain", "v", h, 2)],
                    5: [("pv", h - 1, 3)],
                    6: [("chain", "q", h, 2)],
                    7: [("chain", "v", h, 3)],
                    8: [("chain", "q", h, 3), ("loadw", "v", h + 2)],
                    9: [("chain", "k", h + 1, 0), ("loadw", "q", h + 2)],
                    10: [("chain", "k", h + 1, 1)],
                    11: [("chain", "k", h + 1, 2)],
                    12: [("pv", h, 0)],
                    13: [("chain", "k", h + 1, 3)],
                    14: [("pv", h, 1)],
                    15: [("chain", "q", h + 1, 0)],
                }
                if h == 0:
                    del fillers[0][1]  # q1(h0) ran in the prime
                if h == 2:
                    # wo0..wo2 alias w-pool buffers freed by head-2's chains
                    fillers[0].append(("loadwo", 0))
                    fillers[8].append(("loadwo", 2))
                    fillers[9].append(("loadwo", 1))
            else:
                # v chains front-loaded so the vv transpose queues ahead of
                # most pbm transposes and beats PV(h3, 0) at u10
                fillers = {
                    0: [("chain", "q", h, 1)],
                    1: [("chain", "v", h, 0)],
                    2: [("chain", "v", h, 1)],
                    3: [("pv", h - 1, 2)],
                    4: [("chain", "v", h, 2)],
                    5: [("chain", "v", h, 3)],
                    6: [("chain", "q", h, 2)],
                    7: [("pv", h - 1, 3)],
                    8: [("chain", "q", h, 3), ("loadwo", 3)],
                    10: [("pv", h, 0)],
                    11: [("op", 3)],
                    12: [("op", 3)],
                    13: [("pv", h, 1)],
                    14: [("op", 3)],
                    15: [("op", 3)],
                }
            for u in range(16):
                scores_unit(h, u)
                for item in fillers.get(u, []):
                    if item[0] == "chain":
                        chain(item[1], item[2], item[3])
                    elif item[0] == "pv":
                        if item[1] >= 0:
                            pv_block(item[1], item[2])
                    elif item[0] == "loadw":
                        if item[2] < HPC:
                            load_w(item[1], item[2])
                    elif item[0] == "loadwo":
                        load_wo(item[1])
                    else:
                        take_op(item[1])

        # ---- tail ----
        # Ready op chains go BEFORE each PV block: the PE queue is in-order,
        # so a PV waiting on slot-3's exp-paced transposes must not block
        # chains whose inputs are already complete.
        take_op(16)
        pv_block(3, 2, copy_eng=nc.scalar)
        take_op(16)
        pv_block(3, 3, copy_eng=nc.scalar)
        take_op(len(op_queue))


def _emit_masked(nc, tc, xT, wqT, wkT, wvT, woT, maskf, out):
    """Two-phase variant with additive mask (baseline path; mask is zero in
    the benchmark so this is correctness-only)."""
    with ExitStack() as octx:
        planes = octx.enter_context(tc.tile_pool(name="planes", bufs=1))
        q_sb = [planes.tile([P, S], BF16, tag=f"q{h}", name=f"q{h}") for h in range(HPC)]
        k_sb = [planes.tile([P, S], BF16, tag=f"k{h}", name=f"k{h}") for h in range(HPC)]
        ctx_sb = [planes.tile([P, S], BF16, tag=f"ctx{h}", name=f"ctx{h}")
                  for h in range(HPC)]
        vv_pool = octx.enter_context(tc.tile_pool(name="vv", bufs=1))
        vvs = [vv_pool.tile([P, NNT, P], BF16, tag=f"vv{h}", name=f"vv{h}")
               for h in range(HPC)]

        ps_sc = octx.enter_context(tc.tile_pool(name="ps_sc", bufs=3, space="PSUM"))
        ps_small = octx.enter_context(
            tc.tile_pool(name="ps_small", bufs=2, space="PSUM")
        )

        with ExitStack() as ctx:
            wpool = ctx.enter_context(tc.tile_pool(name="wqkv", bufs=1))
            w_sb = {}
            for name, src in (("q", wqT), ("k", wkT), ("v", wvT)):
                w_sb[name] = wpool.tile([P, NK, DL], BF16, tag=f"w{name}",
                                        name=f"w{name}")
            vT_sb = [wpool.tile([P, S], BF16, tag=f"vt{h}", name=f"vt{h}")
                     for h in range(HPC)]
            xpool = ctx.enter_context(tc.tile_pool(name="xt", bufs=1))
            xts = [xpool.tile([P, S], BF16, tag=f"x{kt}", name=f"x{kt}")
                   for kt in range(NK)]
            for kt in range(NK):
                nc.sync.dma_start(xts[kt][:], xT[kt * P : (kt + 1) * P, :])
            for name, src in (("q", wqT), ("k", wkT), ("v", wvT)):
                for kt in range(NK):
                    nc.gpsimd.dma_start(
                        w_sb[name][:, kt, :], src[kt * P : (kt + 1) * P, :]
                    )

            for h in range(HPC):
                for name, dsts in (("q", q_sb), ("k", k_sb), ("v", vT_sb)):
                    for half in range(2):
                        ps = ps_sc.tile([P, 1024], F32, tag="ps_sc")
                        for kt in range(NK):
                            for mc in range(2):
                                m0 = half * 1024 + mc * 512
                                nc.tensor.matmul(
                                    ps[:, mc * 512 : (mc + 1) * 512],
                                    w_sb[name][:, kt, h * P : (h + 1) * P],
                                    xts[kt][:, m0 : m0 + 512],
                                    start=(kt == 0),
                                    stop=(kt == NK - 1),
                                )
                        nc.any.tensor_copy(
                            dsts[h][:, half * 1024 : (half + 1) * 1024], ps[:]
                        )
                nc.sync.dma_start(vvs[h][:], vT_sb[h][:], transpose=True)

        with ExitStack() as ctx:
            wopool = ctx.enter_context(tc.tile_pool(name="wo", bufs=1))
            wo_sb = [wopool.tile([P, DIM], BF16, tag=f"wo{h}", name=f"wo{h}")
                     for h in range(HPC)]
            for h in range(HPC):
                nc.gpsimd.dma_start(wo_sb[h][:], woT[h * P : (h + 1) * P, :])

            pbm_pool = ctx.enter_context(tc.tile_pool(name="pbm", bufs=14))
            pbt_pool = ctx.enter_context(tc.tile_pool(name="pbt", bufs=4))
            stats = ctx.enter_context(tc.tile_pool(name="stats", bufs=8))
            recp = ctx.enter_context(tc.tile_pool(name="recp", bufs=16))
            ob_pool = ctx.enter_context(tc.tile_pool(name="ob", bufs=3))
            mpool = ctx.enter_context(tc.tile_pool(name="mask", bufs=3))

            def scores_unit(h, jb, mtl, pbt_dst):
                mt = 4 * jb + mtl
                qt = q_sb[h][:, mt * P : (mt + 1) * P]
                pbm = pbm_pool.tile([P, S], BF16, tag="pbm", name="pbm")
                accs = stats.tile([P, 2], F32, tag="accs", name="accs")
                mts = mpool.tile([P, S], F32, tag="mt", name="mts")
                nc.gpsimd.dma_start(mts[:], maskf[mt * P : (mt + 1) * P, :])
                for c in range(2):
                    ps = ps_sc.tile([P, 1024], F32, tag="ps_sc")
                    for sub in range(2):
                        k0 = c * 1024 + sub * 512
                        nc.tensor.matmul(
                            ps[:, sub * 512 : (sub + 1) * 512],
                            qt,
                            k_sb[h][:, k0 : k0 + 512],
                            start=True,
                            stop=True,
                        )
                    nc.vector.tensor_add(
                        ps[:], ps[:], mts[:, c * 1024 : (c + 1) * 1024]
                    )
                    nc.scalar.activation(
                        pbm[:, c * 1024 : (c + 1) * 1024],
                        ps[:],
                        mybir.ActivationFunctionType.Exp,
                        scale=SCALE,
                        accum_out=accs[:, c : c + 1],
                    )
                den = stats.tile([P, 1], F32, tag="den", name="den")
                nc.vector.tensor_add(den[:], accs[:, 0:1], accs[:, 1:2])
                rec = recp.tile([P, 1], F32, tag="rec", name="rec")
                nc.vector.reciprocal(rec[:], den[:])
                nc.vector.tensor_scalar_mul(pbm[:], pbm[:], rec[:])
                nc.sync.dma_start(
                    pbt_dst[:, :, mtl, :], pbm[:], transpose=True
                )

            def make_pv(h, jb, pbt):
                def pv():
                    ps = ps_small.tile([P, 512], F32, tag="ps_small")
                    for nt in range(NNT):
                        nc.tensor.matmul(
                            ps[:],
                            vvs[h][:, nt, :],
                            pbt[:, nt, :, :],
                            start=(nt == 0),
                            stop=(nt == NNT - 1),
                        )
                    nc.vector.tensor_copy(
                        ctx_sb[h][:, jb * 512 : (jb + 1) * 512], ps[:]
                    )

                    def finish():
                        pass

                    return finish
                return pv

            def outproj_chain(tt, ec, store_eng=None):
                ps = ps_small.tile([P, 512], F32, tag="ps_small")
                for h in range(HPC):
                    nc.tensor.matmul(
                        ps[:],
                        ctx_sb[h][:, tt * P : (tt + 1) * P],
                        wo_sb[h][:, ec * 512 : (ec + 1) * 512],
                        start=(h == 0),
                        stop=(h == HPC - 1),
                    )
                ob = ob_pool.tile([P, 512], F32, tag="ob", name="ob")
                nc.vector.tensor_copy(ob[:], ps[:])
                (store_eng or nc.gpsimd).dma_start(
                    out[tt * P : (tt + 1) * P, ec * 512 : (ec + 1) * 512],
                    ob[:],
                )

            TAKES = {9: 1, 10: 1, 11: 1, 12: 1, 13: 1, 14: 1, 15: 2,
                     0: 2, 1: 2, 2: 2, 3: 2}
            TAKES_LAST = {0: 2, 1: 2, 2: 2, 3: 2,
                          8: 2, 9: 2, 10: 2, 11: 2, 12: 2, 13: 2, 14: 2, 15: 2}
            pv_pending = []
            fin2 = []
            op_pending = []
            op_next = []
            for jb in range(4):
                for h in range(HPC):
                    pbt = pbt_pool.tile([P, NNT, 4, P], BF16, tag="pbt",
                                        name="pbt")
                    for mtl in range(4):
                        unit_idx = 4 * h + mtl
                        scores_unit(h, jb, mtl, pbt)
                        if fin2:
                            fin2.pop(0)()
                        takes = TAKES_LAST if jb == 3 else TAKES
                        for _ in range(takes.get(unit_idx, 0)):
                            if op_pending:
                                outproj_chain(*op_pending.pop(0))
                    pv_pending.append(make_pv(h, jb, pbt))
                    if len(pv_pending) > 2:
                        fin2.append(pv_pending.pop(0)())
                    if h == 1 and op_next:
                        op_pending = op_next + op_pending
                        op_next = []
                op_next = [(4 * jb + i, ec) for i in range(4) for ec in range(4)]
            while pv_pending:
                fin2.append(pv_pending.pop(0)())
            while fin2:
                fin2.pop(0)()
            for chains in (op_pending, op_next):
                while chains:
                    outproj_chain(*chains.pop(0))


def _build(with_mask: bool):
    nc = bacc.Bacc("TRN2")
    xT = nc.dram_tensor("xT", [DIM, S], BF16, kind="ExternalInput")
    if with_mask:
        wqT = nc.dram_tensor("wqT", [DIM, DL], BF16, kind="ExternalInput")
        wkT = nc.dram_tensor("wkT", [DIM, DL], BF16, kind="ExternalInput")
        wvT = nc.dram_tensor("wvT", [DIM, DL], BF16, kind="ExternalInput")
        maskf = nc.dram_tensor("maskf", [S, S], F32, kind="ExternalInput")
        woT = nc.dram_tensor("woT", [DL, DIM], BF16, kind="ExternalInput")
        out = nc.dram_tensor("out", [S, DIM], F32, kind="ExternalOutput")
        with tile.TileContext(nc) as tc:
            _emit_masked(nc, tc, xT, wqT, wkT, wvT, woT, maskf, out)
    else:
        wq_arr = nc.dram_tensor("wq_arr", [HPC, P, NK, P], BF16,
                                kind="ExternalInput")
        wk_arr = nc.dram_tensor("wk_arr", [HPC, P, NK, P], BF16,
                                kind="ExternalInput")
        wv_arr = nc.dram_tensor("wv_arr", [HPC, P, NK, P], BF16,
                                kind="ExternalInput")
        woT = nc.dram_tensor("woT", [DL, DIM], BF16, kind="ExternalInput")
        out = nc.dram_tensor("out", [S, DIM], BF16, kind="ExternalOutput")
        with tile.TileContext(nc) as tc:
            _emit_fast(nc, tc, xT, wq_arr, wk_arr, wv_arr, woT, out)
    nc.finalize()
    return nc


def _get_program(with_mask: bool):
    if with_mask not in _PROGRAM_CACHE:
        _PROGRAM_CACHE[with_mask] = _build(with_mask)
    return _PROGRAM_CACHE[with_mask]


def _w_headwise(w, sl):
    """wq[sl,:].T as [HPC, P(part), NK, P] so each head's stationary tiles
    are one contiguous DMA."""
    bf = ml_dtypes.bfloat16
    A = np.asarray(w)[sl, :].T.astype(bf)          # [DIM, DL]
    A = A.reshape(NK, P, HPC, P).transpose(2, 1, 0, 3)
    return np.ascontiguousarray(A)


def _prep_in_maps(x, mask, wq, wk, wv, wo, with_mask):
    bf = ml_dtypes.bfloat16
    f32 = np.float32
    xTs = [np.ascontiguousarray(x[b].T.astype(bf)) for b in range(B)]
    if with_mask:
        maskf = np.ascontiguousarray(mask[0, 0].astype(f32) / SCALE)
    in_maps = []
    for c in range(N_CORES):
        b = c // HGROUPS
        g = c % HGROUPS
        sl = slice(g * DL, (g + 1) * DL)
        if with_mask:
            m = {
                "xT": xTs[b],
                "wqT": np.ascontiguousarray(wq[sl, :].T.astype(bf)),
                "wkT": np.ascontiguousarray(wk[sl, :].T.astype(bf)),
                "wvT": np.ascontiguousarray(wv[sl, :].T.astype(bf)),
                "woT": np.ascontiguousarray(wo[:, sl].T.astype(bf)),
                "maskf": maskf,
            }
        else:
            m = {
                "xT": xTs[b],
                "wq_arr": _w_headwise(wq, sl),
                "wk_arr": _w_headwise(wk, sl),
                "wv_arr": _w_headwise(wv, sl),
                "woT": np.ascontiguousarray(wo[:, sl].T.astype(bf)),
            }
        in_maps.append(m)
    return in_maps


def run_sharded(x, mask, wq, wk, wv, wo, trace=False, trace_kwargs=None):
    """Run the SPMD kernel; returns (full_output, BassKernelResults)."""
    with_mask = bool(np.any(np.asarray(mask)))
    nc = _get_program(with_mask)
    in_maps = _prep_in_maps(
        np.asarray(x), np.asarray(mask), np.asarray(wq), np.asarray(wk),
        np.asarray(wv), np.asarray(wo), with_mask,
    )
    kw = {}
    if trace:
        kw["trace"] = True
        if trace_kwargs:
            kw["trace_kwargs"] = trace_kwargs
    res = run_bass_kernel_spmd(nc, in_maps, list(range(N_CORES)), **kw)
    out = np.zeros((B, S, DIM), np.float32)
    for c in range(N_CORES):
        out[c // HGROUPS] += np.asarray(res.results[c]["out"],
                                        dtype=np.float32)
    return out, res


def kernel(**inputs):
    out, _ = run_sharded(
        inputs["x"], inputs["mask"], inputs["wq"], inputs["wk"], inputs["wv"],
        inputs["wo"],
    )
    return out
